# revision 17
# baseline (speedup 1.0000x reference)
"""Multi-head attention kernel for TRN2, 8 NeuronCores (v4).

Problem: x (8, 256, 32, 32); qkv = w_qkv @ x_flat per batch; q, k l2-normalized
over the token axis; sim = 10 * q^T k; softmax over keys; out = attn @ v^T;
y = w_out @ out_hidden + b_out.

Sharding: pure data-parallel - batch 8 across 8 cores, one batch each.
No collectives. Weights replicated (transposed host-side).

Key structure (vs 137.8us baseline):
  - sim matmuls ROW-PACKED: the two heads of a q/k chunk pair run as
    concurrent K=64 matmuls on disjoint PE row groups (tile_position
    (0,0)/(64,0)).
  - AV+denominator COL-PACKED: per (head, half, jc) two concurrent M=64
    matmuls - all-ones lhsT -> denominator replicated on partitions 0-63,
    v_h lhsT -> out values on partitions 64-127. Tail is just
    reciprocal_approx + one elementwise mul (partition-aligned).
  - exp split: even head of each pair -> ScalarE exact exp, odd head ->
    VectorE fast-exp (int16 bit trick bitcast_bf16(round(S*128*log2e +
    127*128-6))). End-to-end rel err ~1.2e-2 (gate 2e-2).
  - ONE ACT table set for the whole kernel: rqk = 10/sqrt(ssq*ssk) is
    computed as exp(-0.5*ln(0.01*prod)); the activation-table registry is
    trimmed (exp removed from set 0, ln from set 5) so both Ln and Exp
    resolve to `natural_log_exp_and_others` - a single ACT_TABLE_LOAD,
    no mid-kernel reloads.
  - pair-(t+1) q/k projections software-pipelined into pair-t sim slots;
    AV of pair t-1 fills pair t's slots.
"""

import numpy as np
import ml_dtypes

import concourse.bass as bass
import concourse.hw_specs as hw_specs
import concourse.mybir as mybir
import concourse.tile as tile
from concourse import bacc
from concourse.bass_utils import run_bass_kernel_spmd

F32 = mybir.dt.float32
BF16 = mybir.dt.bfloat16
I16 = mybir.dt.int16
AF = mybir.ActivationFunctionType
ALU = mybir.AluOpType

B = 8          # batch (one per core)
C = 256        # input channels
N = 1024       # tokens (32*32)
HID = 512      # heads * dim_head
HEADS = 8
DH = 64
NCORES = 8
XW_COLS = 6144

LOG2E = 1.4426950408889634
FE_A = 128.0 * LOG2E
FE_B = 127.0 * 128.0 - 6.0

_cache = {}


def _unify_act_tables(arch):
    """Make Ln and Exp both resolve to the combined
    `natural_log_exp_and_others` set so the kernel needs exactly one
    ACT_TABLE_LOAD. The set-id <-> position mapping is preserved; we only
    stop the earlier sets from claiming these two functions."""
    tables = hw_specs.get_activation_tables(arch)
    names = list(tables.keys())
    combined = next(n for n in names if "natural_log_exp" in n)
    for name, fns in tables.items():
        if name != combined:
            fns.discard(AF.Exp)
            fns.discard(AF.Ln)
    return tables


def _build():
    nc = bacc.Bacc("TRN2", target_bir_lowering=False, debug=False)
    _unify_act_tables(nc.m.arch)

    xw_d = nc.dram_tensor("xw", [128, XW_COLS], BF16, kind="ExternalInput")
    b_d = nc.dram_tensor("b_out", [C, 1], F32, kind="ExternalInput")
    out_d = nc.dram_tensor("out", [C, N], F32, kind="ExternalOutput")

    with tile.TileContext(nc) as tc:
        _body(nc, tc, xw_d, b_d, out_d)

    nc.compile()
    return nc


def _body(nc, tc, xw_d, b_d, out_d):
    from contextlib import ExitStack

    ctx = ExitStack()
    with ctx:
        const = ctx.enter_context(tc.tile_pool(name="const", bufs=1))
        qkp = ctx.enter_context(tc.tile_pool(name="qkhat", bufs=8))
        sqs = ctx.enter_context(tc.tile_pool(name="sqscr", bufs=2))
        vtp = ctx.enter_context(tc.tile_pool(name="vt1", bufs=8))
        esa = ctx.enter_context(tc.tile_pool(name="esa", bufs=17))
        esb = ctx.enter_context(tc.tile_pool(name="esb", bufs=17))
        ohp = ctx.enter_context(tc.tile_pool(name="outh", bufs=4))
        yp = ctx.enter_context(tc.tile_pool(name="y", bufs=2))
        stat = ctx.enter_context(tc.tile_pool(name="stat", bufs=24))
        recp = ctx.enter_context(tc.tile_pool(name="rec", bufs=4))
        ps = ctx.enter_context(tc.tile_pool(name="ps", bufs=3, space="PSUM"))
        psu = ctx.enter_context(tc.tile_pool(name="psu", bufs=2, space="PSUM"))

        # ---- inputs: packed [xb0|xb1|wqk0|wqk1|wv0|wv1|wout0..3]
        big = const.tile([128, XW_COLS], BF16, tag="big")
        nc.sync.dma_start(big[:, 0:3072], xw_d[:, 0:3072])
        nc.gpsimd.dma_start(big[:, 3072:XW_COLS], xw_d[:, 3072:XW_COLS])
        xb = [big[:, 0:1024], big[:, 1024:2048]]
        wqk = [big[:, 2048:3072], big[:, 3072:4096]]
        wv = [big[:, 4096:4608], big[:, 4608:5120]]
        wout = [big[:, 5120 + c * 256:5120 + (c + 1) * 256] for c in range(4)]
        bias = []
        for c in range(2):
            t = const.tile([128, 1], F32, tag=f"bias{c}")
            nc.gpsimd.dma_start(t[:], b_d[c * 128:(c + 1) * 128, :])
            bias.append(t)
        onescol_f = const.tile([128, HEADS], F32, tag="onescol")
        nc.gpsimd.memset(onescol_f[:], 1.0)
        # bf16 all-ones stationary operand for the denominator matmuls
        ones64 = const.tile([128, DH], BF16, tag="ones64")
        one_bits = float(np.frombuffer(np.uint32(0x3F803F80).tobytes(),
                                       dtype=np.float32)[0])
        nc.gpsimd.memset(ones64[:].bitcast(F32)[:, 0:DH // 2], one_bits)

        # PE warmup junk matmuls during the input DMA window
        wu_w = const.tile([128, 128], BF16, tag="wu_w")
        nc.gpsimd.memset(wu_w[:].bitcast(F32)[:, 0:64], 0.0)
        wu_r = const.tile([128, 512], BF16, tag="wu_r")
        nc.gpsimd.memset(wu_r[:].bitcast(F32)[:, 0:256], 0.0)
        for _ in range(14):
            wj = psu.tile([128, 512], F32, tag="u", name="wu")
            nc.tensor.matmul(wj[:], wu_w[:], wu_r[:])

        # single ACT table set (ln+exp): load during the input-DMA window
        dum = stat.tile([128, 1], F32, tag="dum", name="dum_ln")
        nc.scalar.activation(dum[:], onescol_f[:, 0:1], AF.Ln)

        qhat = [None] * 4
        khat = [None] * 4
        ssq = [None] * 8

        def proj_mms(oc):
            P = ps.tile([128, N], F32, tag="ps", name=f"pqk{oc}")
            for half in range(2):
                sl = slice(half * 512, (half + 1) * 512)
                for kc in range(2):
                    nc.tensor.matmul(
                        P[:, sl],
                        wqk[kc][:, oc * 128:(oc + 1) * 128],
                        xb[kc][:, sl],
                        start=(kc == 0),
                        stop=(kc == 1),
                    )
            return P

        def q_side(c, Pq, dve_evac=False):
            e = qkp.tile([128, N], BF16, tag="qk", name=f"q{c}")
            if dve_evac:
                nc.vector.tensor_copy(e[:], Pq[:])
            else:
                nc.scalar.activation(e[:], Pq[:], AF.Copy)
            sq = sqs.tile([128, N], BF16, tag="sq", name=f"sqq{c}")
            s = stat.tile([128, 1], F32, tag="ssq", name=f"ssq{c}")
            nc.scalar.activation(sq[:], Pq[:], AF.Square, accum_out=s[:])
            qhat[c] = e
            ssq[c] = s

        def k_side(c, Pk, dve_evac=False):
            sq = sqs.tile([128, N], BF16, tag="sq", name=f"sqk{c}")
            s = stat.tile([128, 1], F32, tag="ssq", name=f"ssk{c}")
            nc.scalar.activation(sq[:], Pk[:], AF.Square, accum_out=s[:])
            ssq[4 + c] = s
            prod = stat.tile([128, 1], F32, tag="prod", name=f"prod{c}")
            nc.vector.tensor_mul(prod[:], ssq[c][:], s[:])
            # rqk = 10/sqrt(prod) = exp(-0.5*ln(0.01*prod)); Ln and Exp share
            # one ACT table set (see _unify_act_tables)
            lp = stat.tile([128, 1], F32, tag="lp", name=f"lp{c}")
            nc.scalar.activation(lp[:], prod[:], AF.Ln, scale=0.01)
            rqk = stat.tile([128, 1], F32, tag="rqk", name=f"rqk{c}")
            nc.scalar.activation(rqk[:], lp[:], AF.Exp, scale=-0.5)
            e = qkp.tile([128, N], BF16, tag="qk", name=f"kh{c}")
            if dve_evac:
                nc.vector.tensor_scalar(e[:], Pk[:], rqk[:], None, ALU.mult)
            else:
                nc.scalar.activation(e[:], Pk[:], AF.Identity, scale=rqk[:])
            khat[c] = e

        # ---- pair 0 projections up front; vT 0-1 up front, 2-7 deferred
        # into pair-0 slots
        q_side(0, proj_mms(0))
        k_side(0, proj_mms(4))

        vt1 = []

        def v_proj(jc):
            Pv = psu.tile([128, HID], F32, tag="u", name=f"pv{jc}")
            for kc in range(2):
                nc.tensor.matmul(
                    Pv[:],
                    xb[kc][:, jc * 128:(jc + 1) * 128],
                    wv[kc],
                    start=(kc == 0),
                    stop=(kc == 1),
                )
            t = vtp.tile([128, HID], BF16, tag="vt", name=f"vt{jc}")
            nc.scalar.activation(t[:], Pv[:], AF.Copy)
            vt1.append(t)

        v_proj(0)
        v_proj(1)

        # ---- attention: 4 head-pairs, software-pipelined
        outh = [ohp.tile([128, N], BF16, tag="oh", name=f"oh{i}") for i in range(4)]
        es_of = {}
        U_half = {}

        def tail(tp, lane, half, U):
            ro = lane * DH
            sl = slice(half * 512, (half + 1) * 512)
            rec = recp.tile([DH, 512], F32, tag="rec", name=f"rec{tp}_{lane}{half}")
            nc.vector.reciprocal_approx_fast(rec[:], U[0:DH, :])
            dst = outh[tp][ro:ro + DH, sl]
            nc.vector.tensor_mul(dst, U[DH:128, :], rec[:])

        def av_q1(tp, lane, half):
            # first half of the U accumulation (jc 0-3); allocates U
            h = 2 * tp + lane
            sl = slice(half * 512, (half + 1) * 512)
            U = psu.tile([128, 512], F32, tag="u", name=f"u{tp}_{lane}{half}")
            U_half[(tp, lane, half)] = U
            es = es_of[(tp, lane)]
            for jc in range(4):
                nc.tensor.matmul(
                    U[0:DH, :], ones64[:], es[jc][:, sl],
                    start=(jc == 0), stop=False, tile_position=(0, 0))
                nc.tensor.matmul(
                    U[DH:128, :], vt1[jc][:, h * DH:(h + 1) * DH], es[jc][:, sl],
                    start=(jc == 0), stop=False, tile_position=(0, 64))

        def av_q2(tp, lane, half):
            # second half (jc 4-7) + normalization tail
            h = 2 * tp + lane
            sl = slice(half * 512, (half + 1) * 512)
            U = U_half[(tp, lane, half)]
            es = es_of[(tp, lane)]
            for jc in range(4, 8):
                nc.tensor.matmul(
                    U[0:DH, :], ones64[:], es[jc][:, sl],
                    start=False, stop=(jc == 7), tile_position=(0, 0))
                nc.tensor.matmul(
                    U[DH:128, :], vt1[jc][:, h * DH:(h + 1) * DH], es[jc][:, sl],
                    start=False, stop=(jc == 7), tile_position=(0, 64))
            tail(tp, lane, half, U)

        def av_slot(tp, slot):
            lane = (slot // 2) % 2
            half = slot // 4
            if slot % 2 == 0:
                av_q1(tp, lane, half)
            else:
                av_q2(tp, lane, half)

        for t in range(4):
            qs = qhat[t]
            ks = khat[t]
            es_a, es_b = [], []
            es_of[(t, 0)] = es_a
            es_of[(t, 1)] = es_b
            for jc in range(8):
                jsl = slice(jc * 128, (jc + 1) * 128)
                SA = ps.tile([128, N], F32, tag="ps", name=f"sa{t}_{jc}")
                SB = ps.tile([128, N], F32, tag="ps", name=f"sb{t}_{jc}")
                for half in range(2):
                    sl = slice(half * 512, (half + 1) * 512)
                    nc.tensor.matmul(SA[:, sl], ks[0:DH, jsl], qs[0:DH, sl],
                                     tile_position=(0, 0))
                    nc.tensor.matmul(SB[:, sl], ks[DH:128, jsl], qs[DH:128, sl],
                                     tile_position=(64, 0))
                eA = esa.tile([128, N], BF16, tag="ea", name=f"ea{t}_{jc}")
                nc.scalar.activation(eA[:], SA[:], AF.Exp)
                es_a.append(eA[:])
                eB = esb.tile([128, N], I16, tag="eb", name=f"eb{t}_{jc}")
                nc.vector.tensor_scalar(eB[:], SB[:], FE_A, FE_B,
                                        ALU.mult, ALU.add)
                es_b.append(eB[:].bitcast(BF16))

                if t == 0:
                    if jc < 6:
                        v_proj(jc + 2)
                    wj = psu.tile([128, 512], F32, tag="u", name="wu2")
                    nc.tensor.matmul(wj[:], wu_w[:], wu_r[:])
                elif t < 3:
                    av_slot(t - 1, jc)
                else:
                    # pair 3: AV of pair 2 shares slots with the first
                    # quarter-starts of pair 3's own AV
                    av_slot(2, jc)
                    if jc == 6:
                        av_q1(3, 0, 0)
                    elif jc == 7:
                        av_q1(3, 1, 0)
                if t < 3:
                    if jc == 1:
                        q_side(t + 1, proj_mms(t + 1), dve_evac=(t == 0))
                    elif jc == 3:
                        k_side(t + 1, proj_mms(t + 5), dve_evac=(t == 0))

        # ---- flush: rest of pair 3's AV, out-proj halves interleaved
        def out_proj(half):
            sl = slice(half * 512, (half + 1) * 512)
            for oc in range(2):
                Py = ps.tile([128, 512], F32, tag="ps", name=f"py{oc}_{half}")
                for kc in range(4):
                    nc.tensor.matmul(
                        Py[:],
                        wout[kc][:, oc * 128:(oc + 1) * 128],
                        outh[kc][:, sl],
                        start=(kc == 0),
                        stop=(kc == 3),
                    )
                yt = yp.tile([128, 512], F32, tag="y", name=f"y{oc}_{half}")
                nc.scalar.activation(yt[:], Py[:], AF.Identity, bias=bias[oc][:])
                nc.sync.dma_start(out_d[oc * 128:(oc + 1) * 128, sl], yt[:])

        av_q2(3, 0, 0)
        av_q2(3, 1, 0)
        out_proj(0)
        av_q1(3, 0, 1)
        av_q2(3, 0, 1)
        av_q1(3, 1, 1)
        av_q2(3, 1, 1)
        out_proj(1)


def _get_compiled():
    if "nc" not in _cache:
        _cache["nc"] = _build()
    return _cache["nc"]


def _prep(x, w_qkv, w_out, b_out):
    bf = ml_dtypes.bfloat16
    xs = x.reshape(B, C, N).astype(bf)              # (B, 256, 1024)
    w_qkT = w_qkv[:2 * HID].T.astype(bf)            # (256, 1024)
    w_vT = w_qkv[2 * HID:].T.astype(bf)             # (256, 512)
    w_outT = w_out.T.astype(bf)                     # (512, 256)
    xw = np.empty((B, 128, XW_COLS), dtype=bf)
    for i in range(B):
        xw[i, :, 0:1024] = xs[i, :128]
        xw[i, :, 1024:2048] = xs[i, 128:]
        xw[i, :, 2048:3072] = w_qkT[:128]
        xw[i, :, 3072:4096] = w_qkT[128:]
        xw[i, :, 4096:4608] = w_vT[:128]
        xw[i, :, 4608:5120] = w_vT[128:]
        for c in range(4):
            xw[i, :, 5120 + c * 256:5120 + (c + 1) * 256] = w_outT[c * 128:(c + 1) * 128]
    return {
        "xw": np.ascontiguousarray(xw),
        "b_out": np.ascontiguousarray(b_out.reshape(C, 1), dtype=np.float32),
    }


def kernel(x, w_qkv, w_out, b_out, **kw):
    nc = _get_compiled()
    x = np.asarray(x, dtype=np.float32)
    w_qkv = np.asarray(w_qkv, dtype=np.float32)
    w_out = np.asarray(w_out, dtype=np.float32)
    b_out = np.asarray(b_out, dtype=np.float32)

    p = _prep(x, w_qkv, w_out, b_out)
    in_maps = [
        {"xw": p["xw"][i], "b_out": p["b_out"]}
        for i in range(NCORES)
    ]
    res = run_bass_kernel_spmd(nc, in_maps, list(range(NCORES)))
    y = np.stack([res.results[i]["out"] for i in range(NCORES)])
    return y.reshape(B, C, 32, 32)


# revision 20
# speedup vs baseline: 1.1883x; 1.1883x over previous
"""Multi-head attention kernel for TRN2, 8 NeuronCores (v4).

Problem: x (8, 256, 32, 32); qkv = w_qkv @ x_flat per batch; q, k l2-normalized
over the token axis; sim = 10 * q^T k; softmax over keys; out = attn @ v^T;
y = w_out @ out_hidden + b_out.

Sharding: pure data-parallel - batch 8 across 8 cores, one batch each.
No collectives. Weights replicated (transposed host-side).

Key structure (vs 137.8us baseline):
  - sim matmuls ROW-PACKED: the two heads of a q/k chunk pair run as
    concurrent K=64 matmuls on disjoint PE row groups (tile_position
    (0,0)/(64,0)).
  - AV+denominator COL-PACKED: per (head, half, jc) two concurrent M=64
    matmuls - all-ones lhsT -> denominator replicated on partitions 0-63,
    v_h lhsT -> out values on partitions 64-127. Tail is just
    reciprocal_approx + one elementwise mul (partition-aligned).
  - exp split: even head of each pair -> ScalarE exact exp, odd head ->
    VectorE fast-exp (int16 bit trick bitcast_bf16(round(S*128*log2e +
    127*128-6))). End-to-end rel err ~1.2e-2 (gate 2e-2).
  - ONE ACT table set for the whole kernel: rqk = 10/sqrt(ssq*ssk) is
    computed as exp(-0.5*ln(0.01*prod)); the activation-table registry is
    trimmed (exp removed from set 0, ln from set 5) so both Ln and Exp
    resolve to `natural_log_exp_and_others` - a single ACT_TABLE_LOAD,
    no mid-kernel reloads.
  - pair-(t+1) q/k projections software-pipelined into pair-t sim slots;
    AV of pair t-1 fills pair t's slots.
"""

import numpy as np
import ml_dtypes

import concourse.bass as bass
import concourse.hw_specs as hw_specs
import concourse.mybir as mybir
import concourse.tile as tile
from concourse import bacc
from concourse.bass_utils import run_bass_kernel_spmd

F32 = mybir.dt.float32
BF16 = mybir.dt.bfloat16
I16 = mybir.dt.int16
AF = mybir.ActivationFunctionType
ALU = mybir.AluOpType

B = 8          # batch (one per core)
C = 256        # input channels
N = 1024       # tokens (32*32)
HID = 512      # heads * dim_head
HEADS = 8
DH = 64
NCORES = 8
XW_COLS = 6144

LOG2E = 1.4426950408889634
FE_A = 128.0 * LOG2E
FE_B = 127.0 * 128.0 - 6.0

_cache = {}


def _unify_act_tables(arch):
    """Make Ln and Exp both resolve to the combined
    `natural_log_exp_and_others` set so the kernel needs exactly one
    ACT_TABLE_LOAD. The set-id <-> position mapping is preserved; we only
    stop the earlier sets from claiming these two functions."""
    tables = hw_specs.get_activation_tables(arch)
    names = list(tables.keys())
    combined = next(n for n in names if "natural_log_exp" in n)
    for name, fns in tables.items():
        if name != combined:
            fns.discard(AF.Exp)
            fns.discard(AF.Ln)
    return tables


def _build():
    nc = bacc.Bacc("TRN2", target_bir_lowering=False, debug=False)
    _unify_act_tables(nc.m.arch)

    xw_d = nc.dram_tensor("xw", [128, XW_COLS], BF16, kind="ExternalInput")
    b_d = nc.dram_tensor("b_out", [C, 1], F32, kind="ExternalInput")
    out_d = nc.dram_tensor("out", [C, N], F32, kind="ExternalOutput")

    with tile.TileContext(nc) as tc:
        _body(nc, tc, xw_d, b_d, out_d)

    nc.compile()
    return nc


def _body(nc, tc, xw_d, b_d, out_d):
    from contextlib import ExitStack

    ctx = ExitStack()
    with ctx:
        const = ctx.enter_context(tc.tile_pool(name="const", bufs=1))
        qkp = ctx.enter_context(tc.tile_pool(name="qkhat", bufs=8))
        sqs = ctx.enter_context(tc.tile_pool(name="sqscr", bufs=2))
        vtp = ctx.enter_context(tc.tile_pool(name="vt1", bufs=8))
        esa = ctx.enter_context(tc.tile_pool(name="esa", bufs=20))
        esb = ctx.enter_context(tc.tile_pool(name="esb", bufs=20))
        ohp = ctx.enter_context(tc.tile_pool(name="outh", bufs=4))
        yp = ctx.enter_context(tc.tile_pool(name="y", bufs=2))
        stat = ctx.enter_context(tc.tile_pool(name="stat", bufs=24))
        recp = ctx.enter_context(tc.tile_pool(name="rec", bufs=4))
        ps = ctx.enter_context(tc.tile_pool(name="ps", bufs=3, space="PSUM"))
        psu = ctx.enter_context(tc.tile_pool(name="psu", bufs=2, space="PSUM"))

        # ---- inputs: packed [xb0|xb1|wqk0|wqk1|wv0|wv1|wout0..3]
        big = const.tile([128, XW_COLS], BF16, tag="big")
        nc.sync.dma_start(big[:, 0:3072], xw_d[:, 0:3072])
        nc.gpsimd.dma_start(big[:, 3072:XW_COLS], xw_d[:, 3072:XW_COLS])
        xb = [big[:, 0:1024], big[:, 1024:2048]]
        wqk = [big[:, 2048:3072], big[:, 3072:4096]]
        wv = [big[:, 4096:4608], big[:, 4608:5120]]
        wout = [big[:, 5120 + c * 256:5120 + (c + 1) * 256] for c in range(4)]
        bias = []
        for c in range(2):
            t = const.tile([128, 1], F32, tag=f"bias{c}")
            nc.gpsimd.dma_start(t[:], b_d[c * 128:(c + 1) * 128, :])
            bias.append(t)
        onescol_f = const.tile([128, HEADS], F32, tag="onescol")
        nc.gpsimd.memset(onescol_f[:], 1.0)
        # bf16 all-ones stationary operand for the denominator matmuls
        ones64 = const.tile([128, DH], BF16, tag="ones64")
        one_bits = float(np.frombuffer(np.uint32(0x3F803F80).tobytes(),
                                       dtype=np.float32)[0])
        nc.gpsimd.memset(ones64[:].bitcast(F32)[:, 0:DH // 2], one_bits)

        # PE warmup junk matmuls during the input DMA window
        wu_w = const.tile([128, 128], BF16, tag="wu_w")
        nc.gpsimd.memset(wu_w[:].bitcast(F32)[:, 0:64], 0.0)
        wu_r = const.tile([128, 512], BF16, tag="wu_r")
        nc.gpsimd.memset(wu_r[:].bitcast(F32)[:, 0:256], 0.0)
        for _ in range(14):
            wj = psu.tile([128, 512], F32, tag="u", name="wu")
            nc.tensor.matmul(wj[:], wu_w[:], wu_r[:])

        # single ACT table set (ln+exp): load during the input-DMA window
        dum = stat.tile([128, 1], F32, tag="dum", name="dum_ln")
        nc.scalar.activation(dum[:], onescol_f[:, 0:1], AF.Ln)

        qhat = [None] * 4
        khat = [None] * 4
        ssq = [None] * 8

        def proj_mms(oc):
            P = ps.tile([128, N], F32, tag="ps", name=f"pqk{oc}")
            for half in range(2):
                sl = slice(half * 512, (half + 1) * 512)
                for kc in range(2):
                    nc.tensor.matmul(
                        P[:, sl],
                        wqk[kc][:, oc * 128:(oc + 1) * 128],
                        xb[kc][:, sl],
                        start=(kc == 0),
                        stop=(kc == 1),
                    )
            return P

        def q_side(c, Pq, dve_evac=False):
            e = qkp.tile([128, N], BF16, tag="qk", name=f"q{c}")
            if dve_evac:
                nc.vector.tensor_copy(e[:], Pq[:])
            else:
                nc.scalar.activation(e[:], Pq[:], AF.Copy)
            sq = sqs.tile([128, N], BF16, tag="sq", name=f"sqq{c}")
            s = stat.tile([128, 1], F32, tag="ssq", name=f"ssq{c}")
            nc.scalar.activation(sq[:], Pq[:], AF.Square, accum_out=s[:])
            qhat[c] = e
            ssq[c] = s

        def k_side(c, Pk, dve_evac=False):
            sq = sqs.tile([128, N], BF16, tag="sq", name=f"sqk{c}")
            s = stat.tile([128, 1], F32, tag="ssq", name=f"ssk{c}")
            nc.scalar.activation(sq[:], Pk[:], AF.Square, accum_out=s[:])
            ssq[4 + c] = s
            prod = stat.tile([128, 1], F32, tag="prod", name=f"prod{c}")
            nc.vector.tensor_mul(prod[:], ssq[c][:], s[:])
            # rqk = 10/sqrt(prod) = exp(-0.5*ln(0.01*prod)); Ln and Exp share
            # one ACT table set (see _unify_act_tables)
            lp = stat.tile([128, 1], F32, tag="lp", name=f"lp{c}")
            nc.scalar.activation(lp[:], prod[:], AF.Ln, scale=0.01)
            rqk = stat.tile([128, 1], F32, tag="rqk", name=f"rqk{c}")
            nc.scalar.activation(rqk[:], lp[:], AF.Exp, scale=-0.5)
            e = qkp.tile([128, N], BF16, tag="qk", name=f"kh{c}")
            if dve_evac:
                nc.vector.tensor_scalar(e[:], Pk[:], rqk[:], None, ALU.mult)
            else:
                nc.scalar.activation(e[:], Pk[:], AF.Identity, scale=rqk[:])
            khat[c] = e

        # ---- pair 0 projections + all of vT up front
        q_side(0, proj_mms(0), dve_evac=True)
        k_side(0, proj_mms(4))

        vt1 = []

        def v_proj(jc):
            Pv = psu.tile([128, HID], F32, tag="u", name=f"pv{jc}")
            for kc in range(2):
                nc.tensor.matmul(
                    Pv[:],
                    xb[kc][:, jc * 128:(jc + 1) * 128],
                    wv[kc],
                    start=(kc == 0),
                    stop=(kc == 1),
                )
            t = vtp.tile([128, HID], BF16, tag="vt", name=f"vt{jc}")
            nc.scalar.activation(t[:], Pv[:], AF.Copy)
            vt1.append(t)

        for jc in range(8):
            v_proj(jc)

        # ---- attention: 4 head-pairs, software-pipelined
        outh = [ohp.tile([128, N], BF16, tag="oh", name=f"oh{i}") for i in range(4)]
        es_of = {}
        U_half = {}

        def tail(tp, lane, half, U):
            ro = lane * DH
            sl = slice(half * 512, (half + 1) * 512)
            rec = recp.tile([DH, 512], F32, tag="rec", name=f"rec{tp}_{lane}{half}")
            nc.vector.reciprocal_approx_fast(rec[:], U[0:DH, :])
            dst = outh[tp][ro:ro + DH, sl]
            nc.vector.tensor_mul(dst, U[DH:128, :], rec[:])

        def av_q1(tp, lane, half):
            # first half of the U accumulation (jc 0-3); allocates U
            h = 2 * tp + lane
            sl = slice(half * 512, (half + 1) * 512)
            U = psu.tile([128, 512], F32, tag="u", name=f"u{tp}_{lane}{half}")
            U_half[(tp, lane, half)] = U
            es = es_of[(tp, lane)]
            for jc in range(4):
                nc.tensor.matmul(
                    U[0:DH, :], ones64[:], es[jc][:, sl],
                    start=(jc == 0), stop=False, tile_position=(0, 0))
                nc.tensor.matmul(
                    U[DH:128, :], vt1[jc][:, h * DH:(h + 1) * DH], es[jc][:, sl],
                    start=(jc == 0), stop=False, tile_position=(0, 64))

        def av_q2(tp, lane, half):
            # second half (jc 4-7) + normalization tail
            h = 2 * tp + lane
            sl = slice(half * 512, (half + 1) * 512)
            U = U_half[(tp, lane, half)]
            es = es_of[(tp, lane)]
            for jc in range(4, 8):
                nc.tensor.matmul(
                    U[0:DH, :], ones64[:], es[jc][:, sl],
                    start=False, stop=(jc == 7), tile_position=(0, 0))
                nc.tensor.matmul(
                    U[DH:128, :], vt1[jc][:, h * DH:(h + 1) * DH], es[jc][:, sl],
                    start=False, stop=(jc == 7), tile_position=(0, 64))
            tail(tp, lane, half, U)

        def av_slot(tp, slot):
            lane = (slot // 2) % 2
            half = slot // 4
            if slot % 2 == 0:
                av_q1(tp, lane, half)
            else:
                av_q2(tp, lane, half)

        for t in range(4):
            qs = qhat[t]
            ks = khat[t]
            es_a, es_b = [], []
            es_of[(t, 0)] = es_a
            es_of[(t, 1)] = es_b
            for jc in range(8):
                jsl = slice(jc * 128, (jc + 1) * 128)
                SA = ps.tile([128, N], F32, tag="ps", name=f"sa{t}_{jc}")
                SB = ps.tile([128, N], F32, tag="ps", name=f"sb{t}_{jc}")
                for half in range(2):
                    sl = slice(half * 512, (half + 1) * 512)
                    nc.tensor.matmul(SA[:, sl], ks[0:DH, jsl], qs[0:DH, sl],
                                     tile_position=(0, 0))
                    nc.tensor.matmul(SB[:, sl], ks[DH:128, jsl], qs[DH:128, sl],
                                     tile_position=(64, 0))
                eA = esa.tile([128, N], BF16, tag="ea", name=f"ea{t}_{jc}")
                nc.scalar.activation(eA[:], SA[:], AF.Exp)
                es_a.append(eA[:])
                eB = esb.tile([128, N], I16, tag="eb", name=f"eb{t}_{jc}")
                nc.vector.tensor_scalar(eB[:], SB[:], FE_A, FE_B,
                                        ALU.mult, ALU.add)
                es_b.append(eB[:].bitcast(BF16))

                if t == 0:
                    pass
                elif t < 3:
                    av_slot(t - 1, jc)
                else:
                    # pair 3: AV of pair 2 shares slots with the first
                    # quarter-starts of pair 3's own AV
                    av_slot(2, jc)
                    if jc == 6:
                        av_q1(3, 0, 0)
                    elif jc == 7:
                        av_q1(3, 1, 0)
                if t < 3:
                    if jc == 1:
                        q_side(t + 1, proj_mms(t + 1), dve_evac=(t == 0))
                    elif jc == 3:
                        k_side(t + 1, proj_mms(t + 5), dve_evac=(t == 0))

        # ---- flush: rest of pair 3's AV, out-proj halves interleaved
        def out_proj(half):
            sl = slice(half * 512, (half + 1) * 512)
            for oc in range(2):
                Py = ps.tile([128, 512], F32, tag="ps", name=f"py{oc}_{half}")
                for kc in range(4):
                    nc.tensor.matmul(
                        Py[:],
                        wout[kc][:, oc * 128:(oc + 1) * 128],
                        outh[kc][:, sl],
                        start=(kc == 0),
                        stop=(kc == 3),
                    )
                yt = yp.tile([128, 512], F32, tag="y", name=f"y{oc}_{half}")
                nc.scalar.activation(yt[:], Py[:], AF.Identity, bias=bias[oc][:])
                nc.sync.dma_start(out_d[oc * 128:(oc + 1) * 128, sl], yt[:])

        av_q2(3, 0, 0)
        av_q2(3, 1, 0)
        out_proj(0)
        av_q1(3, 0, 1)
        av_q2(3, 0, 1)
        av_q1(3, 1, 1)
        av_q2(3, 1, 1)
        out_proj(1)


def _get_compiled():
    if "nc" not in _cache:
        _cache["nc"] = _build()
    return _cache["nc"]


def _prep(x, w_qkv, w_out, b_out):
    bf = ml_dtypes.bfloat16
    xs = x.reshape(B, C, N).astype(bf)              # (B, 256, 1024)
    w_qkT = w_qkv[:2 * HID].T.astype(bf)            # (256, 1024)
    w_vT = w_qkv[2 * HID:].T.astype(bf)             # (256, 512)
    w_outT = w_out.T.astype(bf)                     # (512, 256)
    xw = np.empty((B, 128, XW_COLS), dtype=bf)
    for i in range(B):
        xw[i, :, 0:1024] = xs[i, :128]
        xw[i, :, 1024:2048] = xs[i, 128:]
        xw[i, :, 2048:3072] = w_qkT[:128]
        xw[i, :, 3072:4096] = w_qkT[128:]
        xw[i, :, 4096:4608] = w_vT[:128]
        xw[i, :, 4608:5120] = w_vT[128:]
        for c in range(4):
            xw[i, :, 5120 + c * 256:5120 + (c + 1) * 256] = w_outT[c * 128:(c + 1) * 128]
    return {
        "xw": np.ascontiguousarray(xw),
        "b_out": np.ascontiguousarray(b_out.reshape(C, 1), dtype=np.float32),
    }


def kernel(x, w_qkv, w_out, b_out, **kw):
    nc = _get_compiled()
    x = np.asarray(x, dtype=np.float32)
    w_qkv = np.asarray(w_qkv, dtype=np.float32)
    w_out = np.asarray(w_out, dtype=np.float32)
    b_out = np.asarray(b_out, dtype=np.float32)

    p = _prep(x, w_qkv, w_out, b_out)
    in_maps = [
        {"xw": p["xw"][i], "b_out": p["b_out"]}
        for i in range(NCORES)
    ]
    res = run_bass_kernel_spmd(nc, in_maps, list(range(NCORES)))
    y = np.stack([res.results[i]["out"] for i in range(NCORES)])
    return y.reshape(B, C, 32, 32)


# revision 21
# speedup vs baseline: 1.2244x; 1.0303x over previous
"""Multi-head attention kernel for TRN2, 8 NeuronCores (v4).

Problem: x (8, 256, 32, 32); qkv = w_qkv @ x_flat per batch; q, k l2-normalized
over the token axis; sim = 10 * q^T k; softmax over keys; out = attn @ v^T;
y = w_out @ out_hidden + b_out.

Sharding: pure data-parallel - batch 8 across 8 cores, one batch each.
No collectives. Weights replicated (transposed host-side).

Key structure (vs 137.8us baseline):
  - sim matmuls ROW-PACKED: the two heads of a q/k chunk pair run as
    concurrent K=64 matmuls on disjoint PE row groups (tile_position
    (0,0)/(64,0)).
  - AV+denominator COL-PACKED: per (head, half, jc) two concurrent M=64
    matmuls - all-ones lhsT -> denominator replicated on partitions 0-63,
    v_h lhsT -> out values on partitions 64-127. Tail is just
    reciprocal_approx + one elementwise mul (partition-aligned).
  - exp split: even head of each pair -> ScalarE exact exp, odd head ->
    VectorE fast-exp (int16 bit trick bitcast_bf16(round(S*128*log2e +
    127*128-6))). End-to-end rel err ~1.2e-2 (gate 2e-2).
  - ONE ACT table set for the whole kernel: rqk = 10/sqrt(ssq*ssk) is
    computed as exp(-0.5*ln(0.01*prod)); the activation-table registry is
    trimmed (exp removed from set 0, ln from set 5) so both Ln and Exp
    resolve to `natural_log_exp_and_others` - a single ACT_TABLE_LOAD,
    no mid-kernel reloads.
  - pair-(t+1) q/k projections software-pipelined into pair-t sim slots;
    AV of pair t-1 fills pair t's slots.
"""

import numpy as np
import ml_dtypes

import concourse.bass as bass
import concourse.hw_specs as hw_specs
import concourse.mybir as mybir
import concourse.tile as tile
from concourse import bacc
from concourse.bass_utils import run_bass_kernel_spmd

F32 = mybir.dt.float32
BF16 = mybir.dt.bfloat16
I16 = mybir.dt.int16
AF = mybir.ActivationFunctionType
ALU = mybir.AluOpType

B = 8          # batch (one per core)
C = 256        # input channels
N = 1024       # tokens (32*32)
HID = 512      # heads * dim_head
HEADS = 8
DH = 64
NCORES = 8
XW_COLS = 6144

LOG2E = 1.4426950408889634
FE_A = 128.0 * LOG2E
FE_B = 127.0 * 128.0 - 6.0

_cache = {}


def _unify_act_tables(arch):
    """Make Ln and Exp both resolve to the combined
    `natural_log_exp_and_others` set so the kernel needs exactly one
    ACT_TABLE_LOAD. The set-id <-> position mapping is preserved; we only
    stop the earlier sets from claiming these two functions."""
    tables = hw_specs.get_activation_tables(arch)
    names = list(tables.keys())
    combined = next(n for n in names if "natural_log_exp" in n)
    for name, fns in tables.items():
        if name != combined:
            fns.discard(AF.Exp)
            fns.discard(AF.Ln)
    return tables


def _build():
    nc = bacc.Bacc("TRN2", target_bir_lowering=False, debug=False)
    _unify_act_tables(nc.m.arch)

    xw_d = nc.dram_tensor("xw", [128, XW_COLS], BF16, kind="ExternalInput")
    b_d = nc.dram_tensor("b_out", [C, 1], F32, kind="ExternalInput")
    out_d = nc.dram_tensor("out", [C, N], F32, kind="ExternalOutput")

    with tile.TileContext(nc) as tc:
        _body(nc, tc, xw_d, b_d, out_d)

    nc.compile()
    return nc


def _body(nc, tc, xw_d, b_d, out_d):
    from contextlib import ExitStack

    ctx = ExitStack()
    with ctx:
        const = ctx.enter_context(tc.tile_pool(name="const", bufs=1))
        qkp = ctx.enter_context(tc.tile_pool(name="qkhat", bufs=8))
        sqs = ctx.enter_context(tc.tile_pool(name="sqscr", bufs=2))
        vtp = ctx.enter_context(tc.tile_pool(name="vt1", bufs=8))
        esa = ctx.enter_context(tc.tile_pool(name="esa", bufs=20))
        esb = ctx.enter_context(tc.tile_pool(name="esb", bufs=20))
        ohp = ctx.enter_context(tc.tile_pool(name="outh", bufs=4))
        yp = ctx.enter_context(tc.tile_pool(name="y", bufs=2))
        stat = ctx.enter_context(tc.tile_pool(name="stat", bufs=24))
        recp = ctx.enter_context(tc.tile_pool(name="rec", bufs=4))
        ps = ctx.enter_context(tc.tile_pool(name="ps", bufs=3, space="PSUM"))
        psu = ctx.enter_context(tc.tile_pool(name="psu", bufs=2, space="PSUM"))

        # ---- inputs: packed [xb0|xb1|wqk0|wqk1|wv0|wv1|wout0..3]
        big = const.tile([128, XW_COLS], BF16, tag="big")
        nc.sync.dma_start(big[:, 0:3072], xw_d[:, 0:3072])
        nc.gpsimd.dma_start(big[:, 3072:XW_COLS], xw_d[:, 3072:XW_COLS])
        xb = [big[:, 0:1024], big[:, 1024:2048]]
        wqk = [big[:, 2048:3072], big[:, 3072:4096]]
        wv = [big[:, 4096:4608], big[:, 4608:5120]]
        wout = [big[:, 5120 + c * 256:5120 + (c + 1) * 256] for c in range(4)]
        bias = []
        for c in range(2):
            t = const.tile([128, 1], F32, tag=f"bias{c}")
            nc.gpsimd.dma_start(t[:], b_d[c * 128:(c + 1) * 128, :])
            bias.append(t)
        onescol_f = const.tile([128, HEADS], F32, tag="onescol")
        nc.gpsimd.memset(onescol_f[:], 1.0)
        # bf16 all-ones stationary operand for the denominator matmuls
        ones64 = const.tile([128, DH], BF16, tag="ones64")
        one_bits = float(np.frombuffer(np.uint32(0x3F803F80).tobytes(),
                                       dtype=np.float32)[0])
        nc.gpsimd.memset(ones64[:].bitcast(F32)[:, 0:DH // 2], one_bits)

        # PE warmup junk matmuls during the input DMA window
        wu_w = const.tile([128, 128], BF16, tag="wu_w")
        nc.gpsimd.memset(wu_w[:].bitcast(F32)[:, 0:64], 0.0)
        wu_r = const.tile([128, 512], BF16, tag="wu_r")
        nc.gpsimd.memset(wu_r[:].bitcast(F32)[:, 0:256], 0.0)
        for _ in range(14):
            wj = psu.tile([128, 512], F32, tag="u", name="wu")
            nc.tensor.matmul(wj[:], wu_w[:], wu_r[:])

        # single ACT table set (ln+exp): load during the input-DMA window
        dum = stat.tile([128, 1], F32, tag="dum", name="dum_ln")
        nc.scalar.activation(dum[:], onescol_f[:, 0:1], AF.Ln)

        qhat = [None] * 4
        khat = [None] * 4
        ssq = [None] * 8

        def proj_mms(oc):
            P = ps.tile([128, N], F32, tag="ps", name=f"pqk{oc}")
            for half in range(2):
                sl = slice(half * 512, (half + 1) * 512)
                for kc in range(2):
                    nc.tensor.matmul(
                        P[:, sl],
                        wqk[kc][:, oc * 128:(oc + 1) * 128],
                        xb[kc][:, sl],
                        start=(kc == 0),
                        stop=(kc == 1),
                    )
            return P

        def q_side(c, Pq, dve_evac=False):
            e = qkp.tile([128, N], BF16, tag="qk", name=f"q{c}")
            if dve_evac:
                nc.vector.tensor_copy(e[:], Pq[:])
            else:
                nc.scalar.activation(e[:], Pq[:], AF.Copy)
            sq = sqs.tile([128, N], BF16, tag="sq", name=f"sqq{c}")
            s = stat.tile([128, 1], F32, tag="ssq", name=f"ssq{c}")
            nc.scalar.activation(sq[:], Pq[:], AF.Square, accum_out=s[:])
            qhat[c] = e
            ssq[c] = s

        def k_side(c, Pk, dve_evac=False):
            sq = sqs.tile([128, N], BF16, tag="sq", name=f"sqk{c}")
            s = stat.tile([128, 1], F32, tag="ssq", name=f"ssk{c}")
            nc.scalar.activation(sq[:], Pk[:], AF.Square, accum_out=s[:])
            ssq[4 + c] = s
            prod = stat.tile([128, 1], F32, tag="prod", name=f"prod{c}")
            nc.vector.tensor_mul(prod[:], ssq[c][:], s[:])
            # rqk = 10/sqrt(prod) = exp(-0.5*ln(0.01*prod)); Ln and Exp share
            # one ACT table set (see _unify_act_tables)
            lp = stat.tile([128, 1], F32, tag="lp", name=f"lp{c}")
            nc.scalar.activation(lp[:], prod[:], AF.Ln, scale=0.01)
            rqk = stat.tile([128, 1], F32, tag="rqk", name=f"rqk{c}")
            nc.scalar.activation(rqk[:], lp[:], AF.Exp, scale=-0.5)
            e = qkp.tile([128, N], BF16, tag="qk", name=f"kh{c}")
            if dve_evac:
                nc.vector.tensor_scalar(e[:], Pk[:], rqk[:], None, ALU.mult)
            else:
                nc.scalar.activation(e[:], Pk[:], AF.Identity, scale=rqk[:])
            khat[c] = e

        # ---- pair 0 projections + all of vT up front
        q_side(0, proj_mms(0), dve_evac=True)
        k_side(0, proj_mms(4))

        vt1 = []

        def v_proj(jc):
            Pv = psu.tile([128, HID], F32, tag="u", name=f"pv{jc}")
            for kc in range(2):
                nc.tensor.matmul(
                    Pv[:],
                    xb[kc][:, jc * 128:(jc + 1) * 128],
                    wv[kc],
                    start=(kc == 0),
                    stop=(kc == 1),
                )
            t = vtp.tile([128, HID], BF16, tag="vt", name=f"vt{jc}")
            if jc % 2 == 1:
                # DVE is idle during startup; alternating engines lets the
                # Pv psum rotation drain twice as fast
                nc.vector.tensor_copy(t[:], Pv[:])
            else:
                nc.scalar.activation(t[:], Pv[:], AF.Copy)
            vt1.append(t)

        for jc in range(8):
            v_proj(jc)

        # ---- attention: 4 head-pairs, software-pipelined
        outh = [ohp.tile([128, N], BF16, tag="oh", name=f"oh{i}") for i in range(4)]
        es_of = {}
        U_half = {}

        def tail(tp, lane, half, U):
            ro = lane * DH
            sl = slice(half * 512, (half + 1) * 512)
            rec = recp.tile([DH, 512], F32, tag="rec", name=f"rec{tp}_{lane}{half}")
            nc.vector.reciprocal_approx_fast(rec[:], U[0:DH, :])
            dst = outh[tp][ro:ro + DH, sl]
            nc.vector.tensor_mul(dst, U[DH:128, :], rec[:])

        def av_q1(tp, lane, half):
            # first half of the U accumulation (jc 0-3); allocates U
            h = 2 * tp + lane
            sl = slice(half * 512, (half + 1) * 512)
            U = psu.tile([128, 512], F32, tag="u", name=f"u{tp}_{lane}{half}")
            U_half[(tp, lane, half)] = U
            es = es_of[(tp, lane)]
            for jc in range(4):
                nc.tensor.matmul(
                    U[0:DH, :], ones64[:], es[jc][:, sl],
                    start=(jc == 0), stop=False, tile_position=(0, 0))
                nc.tensor.matmul(
                    U[DH:128, :], vt1[jc][:, h * DH:(h + 1) * DH], es[jc][:, sl],
                    start=(jc == 0), stop=False, tile_position=(0, 64))

        def av_q2(tp, lane, half):
            # second half (jc 4-7) + normalization tail
            h = 2 * tp + lane
            sl = slice(half * 512, (half + 1) * 512)
            U = U_half[(tp, lane, half)]
            es = es_of[(tp, lane)]
            for jc in range(4, 8):
                nc.tensor.matmul(
                    U[0:DH, :], ones64[:], es[jc][:, sl],
                    start=False, stop=(jc == 7), tile_position=(0, 0))
                nc.tensor.matmul(
                    U[DH:128, :], vt1[jc][:, h * DH:(h + 1) * DH], es[jc][:, sl],
                    start=False, stop=(jc == 7), tile_position=(0, 64))
            tail(tp, lane, half, U)

        def av_slot(tp, slot):
            lane = (slot // 2) % 2
            half = slot // 4
            if slot % 2 == 0:
                av_q1(tp, lane, half)
            else:
                av_q2(tp, lane, half)

        for t in range(4):
            qs = qhat[t]
            ks = khat[t]
            es_a, es_b = [], []
            es_of[(t, 0)] = es_a
            es_of[(t, 1)] = es_b
            for jc in range(8):
                jsl = slice(jc * 128, (jc + 1) * 128)
                SA = ps.tile([128, N], F32, tag="ps", name=f"sa{t}_{jc}")
                SB = ps.tile([128, N], F32, tag="ps", name=f"sb{t}_{jc}")
                for half in range(2):
                    sl = slice(half * 512, (half + 1) * 512)
                    nc.tensor.matmul(SA[:, sl], ks[0:DH, jsl], qs[0:DH, sl],
                                     tile_position=(0, 0))
                    nc.tensor.matmul(SB[:, sl], ks[DH:128, jsl], qs[DH:128, sl],
                                     tile_position=(64, 0))
                eA = esa.tile([128, N], BF16, tag="ea", name=f"ea{t}_{jc}")
                nc.scalar.activation(eA[:], SA[:], AF.Exp)
                es_a.append(eA[:])
                eB = esb.tile([128, N], I16, tag="eb", name=f"eb{t}_{jc}")
                nc.vector.tensor_scalar(eB[:], SB[:], FE_A, FE_B,
                                        ALU.mult, ALU.add)
                es_b.append(eB[:].bitcast(BF16))

                if t == 0:
                    pass
                elif t < 3:
                    av_slot(t - 1, jc)
                else:
                    # pair 3: AV of pair 2 shares slots with the first
                    # quarter-starts of pair 3's own AV
                    av_slot(2, jc)
                    if jc == 6:
                        av_q1(3, 0, 0)
                    elif jc == 7:
                        av_q1(3, 1, 0)
                if t < 3:
                    if jc == 1:
                        q_side(t + 1, proj_mms(t + 1), dve_evac=(t == 0))
                    elif jc == 3:
                        k_side(t + 1, proj_mms(t + 5), dve_evac=(t == 0))

        # ---- flush: rest of pair 3's AV, out-proj halves interleaved
        def out_proj(half):
            sl = slice(half * 512, (half + 1) * 512)
            for oc in range(2):
                Py = ps.tile([128, 512], F32, tag="ps", name=f"py{oc}_{half}")
                for kc in range(4):
                    nc.tensor.matmul(
                        Py[:],
                        wout[kc][:, oc * 128:(oc + 1) * 128],
                        outh[kc][:, sl],
                        start=(kc == 0),
                        stop=(kc == 3),
                    )
                yt = yp.tile([128, 512], F32, tag="y", name=f"y{oc}_{half}")
                nc.scalar.activation(yt[:], Py[:], AF.Identity, bias=bias[oc][:])
                nc.sync.dma_start(out_d[oc * 128:(oc + 1) * 128, sl], yt[:])

        av_q2(3, 0, 0)
        av_q2(3, 1, 0)
        out_proj(0)
        av_q1(3, 0, 1)
        av_q2(3, 0, 1)
        av_q1(3, 1, 1)
        av_q2(3, 1, 1)
        out_proj(1)


def _get_compiled():
    if "nc" not in _cache:
        _cache["nc"] = _build()
    return _cache["nc"]


def _prep(x, w_qkv, w_out, b_out):
    bf = ml_dtypes.bfloat16
    xs = x.reshape(B, C, N).astype(bf)              # (B, 256, 1024)
    w_qkT = w_qkv[:2 * HID].T.astype(bf)            # (256, 1024)
    w_vT = w_qkv[2 * HID:].T.astype(bf)             # (256, 512)
    w_outT = w_out.T.astype(bf)                     # (512, 256)
    xw = np.empty((B, 128, XW_COLS), dtype=bf)
    for i in range(B):
        xw[i, :, 0:1024] = xs[i, :128]
        xw[i, :, 1024:2048] = xs[i, 128:]
        xw[i, :, 2048:3072] = w_qkT[:128]
        xw[i, :, 3072:4096] = w_qkT[128:]
        xw[i, :, 4096:4608] = w_vT[:128]
        xw[i, :, 4608:5120] = w_vT[128:]
        for c in range(4):
            xw[i, :, 5120 + c * 256:5120 + (c + 1) * 256] = w_outT[c * 128:(c + 1) * 128]
    return {
        "xw": np.ascontiguousarray(xw),
        "b_out": np.ascontiguousarray(b_out.reshape(C, 1), dtype=np.float32),
    }


def kernel(x, w_qkv, w_out, b_out, **kw):
    nc = _get_compiled()
    x = np.asarray(x, dtype=np.float32)
    w_qkv = np.asarray(w_qkv, dtype=np.float32)
    w_out = np.asarray(w_out, dtype=np.float32)
    b_out = np.asarray(b_out, dtype=np.float32)

    p = _prep(x, w_qkv, w_out, b_out)
    in_maps = [
        {"xw": p["xw"][i], "b_out": p["b_out"]}
        for i in range(NCORES)
    ]
    res = run_bass_kernel_spmd(nc, in_maps, list(range(NCORES)))
    y = np.stack([res.results[i]["out"] for i in range(NCORES)])
    return y.reshape(B, C, 32, 32)


# revision 22
# speedup vs baseline: 1.2269x; 1.0021x over previous
"""Multi-head attention kernel for TRN2, 8 NeuronCores (v4).

Problem: x (8, 256, 32, 32); qkv = w_qkv @ x_flat per batch; q, k l2-normalized
over the token axis; sim = 10 * q^T k; softmax over keys; out = attn @ v^T;
y = w_out @ out_hidden + b_out.

Sharding: pure data-parallel - batch 8 across 8 cores, one batch each.
No collectives. Weights replicated (transposed host-side).

Key structure (vs 137.8us baseline):
  - sim matmuls ROW-PACKED: the two heads of a q/k chunk pair run as
    concurrent K=64 matmuls on disjoint PE row groups (tile_position
    (0,0)/(64,0)).
  - AV+denominator COL-PACKED: per (head, half, jc) two concurrent M=64
    matmuls - all-ones lhsT -> denominator replicated on partitions 0-63,
    v_h lhsT -> out values on partitions 64-127. Tail is just
    reciprocal_approx + one elementwise mul (partition-aligned).
  - exp split: even head of each pair -> ScalarE exact exp, odd head ->
    VectorE fast-exp (int16 bit trick bitcast_bf16(round(S*128*log2e +
    127*128-6))). End-to-end rel err ~1.2e-2 (gate 2e-2).
  - ONE ACT table set for the whole kernel: rqk = 10/sqrt(ssq*ssk) is
    computed as exp(-0.5*ln(0.01*prod)); the activation-table registry is
    trimmed (exp removed from set 0, ln from set 5) so both Ln and Exp
    resolve to `natural_log_exp_and_others` - a single ACT_TABLE_LOAD,
    no mid-kernel reloads.
  - pair-(t+1) q/k projections software-pipelined into pair-t sim slots;
    AV of pair t-1 fills pair t's slots.
"""

import numpy as np
import ml_dtypes

import concourse.bass as bass
import concourse.hw_specs as hw_specs
import concourse.mybir as mybir
import concourse.tile as tile
from concourse import bacc
from concourse.bass_utils import run_bass_kernel_spmd

F32 = mybir.dt.float32
BF16 = mybir.dt.bfloat16
I16 = mybir.dt.int16
AF = mybir.ActivationFunctionType
ALU = mybir.AluOpType

B = 8          # batch (one per core)
C = 256        # input channels
N = 1024       # tokens (32*32)
HID = 512      # heads * dim_head
HEADS = 8
DH = 64
NCORES = 8
XW_COLS = 6144

LOG2E = 1.4426950408889634
FE_A = 128.0 * LOG2E
FE_B = 127.0 * 128.0 - 6.0

_cache = {}


def _unify_act_tables(arch):
    """Make Ln and Exp both resolve to the combined
    `natural_log_exp_and_others` set so the kernel needs exactly one
    ACT_TABLE_LOAD. The set-id <-> position mapping is preserved; we only
    stop the earlier sets from claiming these two functions."""
    tables = hw_specs.get_activation_tables(arch)
    names = list(tables.keys())
    combined = next(n for n in names if "natural_log_exp" in n)
    for name, fns in tables.items():
        if name != combined:
            fns.discard(AF.Exp)
            fns.discard(AF.Ln)
    return tables


def _build():
    nc = bacc.Bacc("TRN2", target_bir_lowering=False, debug=False)
    _unify_act_tables(nc.m.arch)

    xw_d = nc.dram_tensor("xw", [128, XW_COLS], BF16, kind="ExternalInput")
    b_d = nc.dram_tensor("b_out", [C, 1], F32, kind="ExternalInput")
    out_d = nc.dram_tensor("out", [C, N], F32, kind="ExternalOutput")

    with tile.TileContext(nc) as tc:
        _body(nc, tc, xw_d, b_d, out_d)

    nc.compile()
    return nc


def _body(nc, tc, xw_d, b_d, out_d):
    from contextlib import ExitStack

    ctx = ExitStack()
    with ctx:
        const = ctx.enter_context(tc.tile_pool(name="const", bufs=1))
        qkp = ctx.enter_context(tc.tile_pool(name="qkhat", bufs=8))
        sqs = ctx.enter_context(tc.tile_pool(name="sqscr", bufs=2))
        vtp = ctx.enter_context(tc.tile_pool(name="vt1", bufs=8))
        esa = ctx.enter_context(tc.tile_pool(name="esa", bufs=20))
        esb = ctx.enter_context(tc.tile_pool(name="esb", bufs=20))
        ohp = ctx.enter_context(tc.tile_pool(name="outh", bufs=4))
        yp = ctx.enter_context(tc.tile_pool(name="y", bufs=2))
        stat = ctx.enter_context(tc.tile_pool(name="stat", bufs=24))
        recp = ctx.enter_context(tc.tile_pool(name="rec", bufs=4))
        ps = ctx.enter_context(tc.tile_pool(name="ps", bufs=3, space="PSUM"))
        psu = ctx.enter_context(tc.tile_pool(name="psu", bufs=2, space="PSUM"))

        # ---- inputs: packed [xb0|xb1|wqk0|wqk1|wv0|wv1|wout0..3]
        big = const.tile([128, XW_COLS], BF16, tag="big")
        nc.sync.dma_start(big[:, 0:3072], xw_d[:, 0:3072])
        nc.gpsimd.dma_start(big[:, 3072:XW_COLS], xw_d[:, 3072:XW_COLS])
        xb = [big[:, 0:1024], big[:, 1024:2048]]
        wqk = [big[:, 2048:3072], big[:, 3072:4096]]
        wv = [big[:, 4096:4608], big[:, 4608:5120]]
        wout = [big[:, 5120 + c * 256:5120 + (c + 1) * 256] for c in range(4)]
        bias = []
        for c in range(2):
            t = const.tile([128, 1], F32, tag=f"bias{c}")
            nc.gpsimd.dma_start(t[:], b_d[c * 128:(c + 1) * 128, :])
            bias.append(t)
        onescol_f = const.tile([128, HEADS], F32, tag="onescol")
        nc.gpsimd.memset(onescol_f[:], 1.0)
        # bf16 all-ones stationary operand for the denominator matmuls
        ones64 = const.tile([128, DH], BF16, tag="ones64")
        one_bits = float(np.frombuffer(np.uint32(0x3F803F80).tobytes(),
                                       dtype=np.float32)[0])
        nc.gpsimd.memset(ones64[:].bitcast(F32)[:, 0:DH // 2], one_bits)

        # PE warmup junk matmuls during the input DMA window
        wu_w = const.tile([128, 128], BF16, tag="wu_w")
        nc.gpsimd.memset(wu_w[:].bitcast(F32)[:, 0:64], 0.0)
        wu_r = const.tile([128, 512], BF16, tag="wu_r")
        nc.gpsimd.memset(wu_r[:].bitcast(F32)[:, 0:256], 0.0)
        for _ in range(14):
            wj = psu.tile([128, 512], F32, tag="u", name="wu")
            nc.tensor.matmul(wj[:], wu_w[:], wu_r[:])

        # single ACT table set (ln+exp): load during the input-DMA window
        dum = stat.tile([128, 1], F32, tag="dum", name="dum_ln")
        nc.scalar.activation(dum[:], onescol_f[:, 0:1], AF.Ln)

        qhat = [None] * 4
        khat = [None] * 4
        ssq = [None] * 8

        def proj_mms(oc):
            P = ps.tile([128, N], F32, tag="ps", name=f"pqk{oc}")
            for half in range(2):
                sl = slice(half * 512, (half + 1) * 512)
                for kc in range(2):
                    nc.tensor.matmul(
                        P[:, sl],
                        wqk[kc][:, oc * 128:(oc + 1) * 128],
                        xb[kc][:, sl],
                        start=(kc == 0),
                        stop=(kc == 1),
                    )
            return P

        def q_side(c, Pq, dve_evac=False):
            e = qkp.tile([128, N], BF16, tag="qk", name=f"q{c}")
            if dve_evac:
                nc.vector.tensor_copy(e[:], Pq[:])
            else:
                nc.scalar.activation(e[:], Pq[:], AF.Copy)
            sq = sqs.tile([128, N], BF16, tag="sq", name=f"sqq{c}")
            s = stat.tile([128, 1], F32, tag="ssq", name=f"ssq{c}")
            nc.scalar.activation(sq[:], Pq[:], AF.Square, accum_out=s[:])
            qhat[c] = e
            ssq[c] = s

        def k_side(c, Pk, dve_evac=False):
            sq = sqs.tile([128, N], BF16, tag="sq", name=f"sqk{c}")
            s = stat.tile([128, 1], F32, tag="ssq", name=f"ssk{c}")
            nc.scalar.activation(sq[:], Pk[:], AF.Square, accum_out=s[:])
            ssq[4 + c] = s
            prod = stat.tile([128, 1], F32, tag="prod", name=f"prod{c}")
            nc.vector.tensor_mul(prod[:], ssq[c][:], s[:])
            # rqk = 10/sqrt(prod) = exp(-0.5*ln(0.01*prod)); Ln and Exp share
            # one ACT table set (see _unify_act_tables)
            lp = stat.tile([128, 1], F32, tag="lp", name=f"lp{c}")
            nc.scalar.activation(lp[:], prod[:], AF.Ln, scale=0.01)
            rqk = stat.tile([128, 1], F32, tag="rqk", name=f"rqk{c}")
            nc.scalar.activation(rqk[:], lp[:], AF.Exp, scale=-0.5)
            e = qkp.tile([128, N], BF16, tag="qk", name=f"kh{c}")
            if dve_evac:
                nc.vector.tensor_scalar(e[:], Pk[:], rqk[:], None, ALU.mult)
            else:
                nc.scalar.activation(e[:], Pk[:], AF.Identity, scale=rqk[:])
            khat[c] = e

        # ---- pair 0 projections + all of vT up front
        q_side(0, proj_mms(0), dve_evac=True)
        k_side(0, proj_mms(4))

        vt1 = []

        def v_proj(jc):
            Pv = psu.tile([128, HID], F32, tag="u", name=f"pv{jc}")
            for kc in range(2):
                nc.tensor.matmul(
                    Pv[:],
                    xb[kc][:, jc * 128:(jc + 1) * 128],
                    wv[kc],
                    start=(kc == 0),
                    stop=(kc == 1),
                )
            t = vtp.tile([128, HID], BF16, tag="vt", name=f"vt{jc}")
            if jc % 2 == 1:
                # DVE is idle during startup; alternating engines lets the
                # Pv psum rotation drain twice as fast
                nc.vector.tensor_copy(t[:], Pv[:])
            else:
                nc.scalar.activation(t[:], Pv[:], AF.Copy)
            vt1.append(t)

        for jc in range(8):
            v_proj(jc)

        # ---- attention: 4 head-pairs, software-pipelined
        outh = [ohp.tile([128, N], BF16, tag="oh", name=f"oh{i}") for i in range(4)]
        es_of = {}
        U_half = {}

        def tail(tp, lane, half, U):
            ro = lane * DH
            sl = slice(half * 512, (half + 1) * 512)
            rec = recp.tile([DH, 512], F32, tag="rec", name=f"rec{tp}_{lane}{half}")
            nc.vector.reciprocal_approx_fast(rec[:], U[0:DH, :])
            dst = outh[tp][ro:ro + DH, sl]
            nc.vector.tensor_mul(dst, U[DH:128, :], rec[:])

        def av_q1(tp, lane, half):
            # first half of the U accumulation (jc 0-3); allocates U
            h = 2 * tp + lane
            sl = slice(half * 512, (half + 1) * 512)
            U = psu.tile([128, 512], F32, tag="u", name=f"u{tp}_{lane}{half}")
            U_half[(tp, lane, half)] = U
            es = es_of[(tp, lane)]
            for jc in range(4):
                nc.tensor.matmul(
                    U[0:DH, :], ones64[:], es[jc][:, sl],
                    start=(jc == 0), stop=False, tile_position=(0, 0))
                nc.tensor.matmul(
                    U[DH:128, :], vt1[jc][:, h * DH:(h + 1) * DH], es[jc][:, sl],
                    start=(jc == 0), stop=False, tile_position=(0, 64))

        def av_q2(tp, lane, half):
            # second half (jc 4-7) + normalization tail
            h = 2 * tp + lane
            sl = slice(half * 512, (half + 1) * 512)
            U = U_half[(tp, lane, half)]
            es = es_of[(tp, lane)]
            for jc in range(4, 8):
                nc.tensor.matmul(
                    U[0:DH, :], ones64[:], es[jc][:, sl],
                    start=False, stop=(jc == 7), tile_position=(0, 0))
                nc.tensor.matmul(
                    U[DH:128, :], vt1[jc][:, h * DH:(h + 1) * DH], es[jc][:, sl],
                    start=False, stop=(jc == 7), tile_position=(0, 64))
            tail(tp, lane, half, U)

        def av_slot(tp, slot):
            lane = (slot // 2) % 2
            half = slot // 4
            if slot % 2 == 0:
                av_q1(tp, lane, half)
            else:
                av_q2(tp, lane, half)

        for t in range(4):
            qs = qhat[t]
            ks = khat[t]
            es_a, es_b = [], []
            es_of[(t, 0)] = es_a
            es_of[(t, 1)] = es_b
            for jc in range(8):
                jsl = slice(jc * 128, (jc + 1) * 128)
                SA = ps.tile([128, N], F32, tag="ps", name=f"sa{t}_{jc}")
                SB = ps.tile([128, N], F32, tag="ps", name=f"sb{t}_{jc}")
                for half in range(2):
                    sl = slice(half * 512, (half + 1) * 512)
                    nc.tensor.matmul(SA[:, sl], ks[0:DH, jsl], qs[0:DH, sl],
                                     tile_position=(0, 0))
                    nc.tensor.matmul(SB[:, sl], ks[DH:128, jsl], qs[DH:128, sl],
                                     tile_position=(64, 0))
                eA = esa.tile([128, N], BF16, tag="ea", name=f"ea{t}_{jc}")
                nc.scalar.activation(eA[:], SA[:], AF.Exp)
                es_a.append(eA[:])
                if t >= 1 and jc in (3, 7):
                    # ScalarE has slack mid-kernel; exact exp also helps accuracy
                    eB = esb.tile([128, N], BF16, tag="eb", name=f"eb{t}_{jc}")
                    nc.scalar.activation(eB[:], SB[:], AF.Exp)
                    es_b.append(eB[:])
                else:
                    eB = esb.tile([128, N], I16, tag="eb", name=f"eb{t}_{jc}")
                    nc.vector.tensor_scalar(eB[:], SB[:], FE_A, FE_B,
                                            ALU.mult, ALU.add)
                    es_b.append(eB[:].bitcast(BF16))

                if t == 0:
                    pass
                elif t < 3:
                    av_slot(t - 1, jc)
                else:
                    # pair 3: AV of pair 2 shares slots with the first
                    # quarter-starts of pair 3's own AV
                    av_slot(2, jc)
                    if jc == 6:
                        av_q1(3, 0, 0)
                    elif jc == 7:
                        av_q1(3, 1, 0)
                if t < 3:
                    if jc == 1:
                        q_side(t + 1, proj_mms(t + 1), dve_evac=(t == 0))
                    elif jc == 3:
                        k_side(t + 1, proj_mms(t + 5), dve_evac=(t == 0))

        # ---- flush: rest of pair 3's AV, out-proj halves interleaved
        def out_proj(half):
            sl = slice(half * 512, (half + 1) * 512)
            for oc in range(2):
                Py = ps.tile([128, 512], F32, tag="ps", name=f"py{oc}_{half}")
                for kc in range(4):
                    nc.tensor.matmul(
                        Py[:],
                        wout[kc][:, oc * 128:(oc + 1) * 128],
                        outh[kc][:, sl],
                        start=(kc == 0),
                        stop=(kc == 3),
                    )
                yt = yp.tile([128, 512], F32, tag="y", name=f"y{oc}_{half}")
                nc.scalar.activation(yt[:], Py[:], AF.Identity, bias=bias[oc][:])
                nc.sync.dma_start(out_d[oc * 128:(oc + 1) * 128, sl], yt[:])

        av_q2(3, 0, 0)
        av_q2(3, 1, 0)
        out_proj(0)
        av_q1(3, 0, 1)
        av_q2(3, 0, 1)
        av_q1(3, 1, 1)
        av_q2(3, 1, 1)
        out_proj(1)


def _get_compiled():
    if "nc" not in _cache:
        _cache["nc"] = _build()
    return _cache["nc"]


def _prep(x, w_qkv, w_out, b_out):
    bf = ml_dtypes.bfloat16
    xs = x.reshape(B, C, N).astype(bf)              # (B, 256, 1024)
    w_qkT = w_qkv[:2 * HID].T.astype(bf)            # (256, 1024)
    w_vT = w_qkv[2 * HID:].T.astype(bf)             # (256, 512)
    w_outT = w_out.T.astype(bf)                     # (512, 256)
    xw = np.empty((B, 128, XW_COLS), dtype=bf)
    for i in range(B):
        xw[i, :, 0:1024] = xs[i, :128]
        xw[i, :, 1024:2048] = xs[i, 128:]
        xw[i, :, 2048:3072] = w_qkT[:128]
        xw[i, :, 3072:4096] = w_qkT[128:]
        xw[i, :, 4096:4608] = w_vT[:128]
        xw[i, :, 4608:5120] = w_vT[128:]
        for c in range(4):
            xw[i, :, 5120 + c * 256:5120 + (c + 1) * 256] = w_outT[c * 128:(c + 1) * 128]
    return {
        "xw": np.ascontiguousarray(xw),
        "b_out": np.ascontiguousarray(b_out.reshape(C, 1), dtype=np.float32),
    }


def kernel(x, w_qkv, w_out, b_out, **kw):
    nc = _get_compiled()
    x = np.asarray(x, dtype=np.float32)
    w_qkv = np.asarray(w_qkv, dtype=np.float32)
    w_out = np.asarray(w_out, dtype=np.float32)
    b_out = np.asarray(b_out, dtype=np.float32)

    p = _prep(x, w_qkv, w_out, b_out)
    in_maps = [
        {"xw": p["xw"][i], "b_out": p["b_out"]}
        for i in range(NCORES)
    ]
    res = run_bass_kernel_spmd(nc, in_maps, list(range(NCORES)))
    y = np.stack([res.results[i]["out"] for i in range(NCORES)])
    return y.reshape(B, C, 32, 32)
